# revision 114
# baseline (speedup 1.0000x reference)
"""Trainium2 Bass kernel for nn_AttentionBlock (cross-attention block + FFN).

Sharding: data-parallel over batch B=8 across the 8 NeuronCores (one batch
element per core). Each core computes the full attention block for its batch
element:
    q = x@Wq+bq, k = cross@Wk+bk, v = cross@Wv+bv        (per-head E=128)
    A = softmax(q k^T / sqrt(E)); out = A v
    x1 = LN(out@Wo + bo + x)
    y  = relu(x1@W1+b1)@W2 + b2
    ret = LN(x1 + y), A

All matmuls run in bf16 on the PE array with fp32 PSUM accumulation;
softmax/layernorm statistics and both outputs stay fp32.
"""

import sys

sys.path.insert(0, "/opt/trn_rl_repo")

import numpy as np

import concourse.bass as bass
import concourse.mybir as mybir
import concourse.tile as tile
from concourse.bass_utils import run_bass_kernel_spmd
from concourse.masks import make_identity
from concourse.tile import ScopedClock

P = 128
L = 1024
S = 1024
D = 1024
H = 8
E = 128
HE = 1024
DFF = 4096
NLT = L // P     # 8 l-tiles
NST = S // P     # 8 s-tiles
NDC = D // P     # 8 d-chunks
NFC = DFF // P   # 32 ff-chunks
NH = 2           # 512-wide halves of a 1024 free dim
FD = 512         # matmul free dim (one PSUM bank of fp32)
H_SPLIT = 6      # layout-A heads computed in the upfront block
SCALE = float(1.0 / np.sqrt(E))
EPS = 1e-5

f32 = mybir.dt.float32
bf16 = mybir.dt.bfloat16
AF = mybir.ActivationFunctionType
ALU = mybir.AluOpType


def _patch_tile_drain():
    """This walrus build rejects >1 sync-wait on the CTRL Drain instruction
    emitted at TileContext exit; split the waits across several drains."""
    if getattr(tile.TileContext, "_drain_patched", False):
        return

    def _drain_and_barrier(self, tick_clock, wait_clock):
        drain_inst = self.nc.sync.drain()
        wait_clock.add_sem_waits(
            drain_inst.ins, ScopedClock({None: tick_clock.global_clock})
        )
        si = drain_inst.ins.sync_info
        waits = list(si.on_wait) if si and si.on_wait else []
        if len(waits) > 1:
            si.on_wait[:] = waits[:1]
            for w in waits[1:]:
                d2 = self.nc.sync.drain()
                si2 = d2.ins.sync_info
                if si2 is None:
                    d2.ins.sync_info = mybir.SyncInfo(on_wait=[w], on_update=[])
                else:
                    si2.on_wait[:] = [w]
        self.nc.all_engine_barrier()
        assert self.sems is not None
        popped = self.nc._tile_sem_poison_stack.pop()
        assert popped is self._sem_poison
        self.nc.clear_and_free_semaphores(list(self.sems.allocated().values()))
        self.nc.all_engine_barrier()

    tile.TileContext._drain_and_barrier = _drain_and_barrier
    tile.TileContext._drain_patched = True


# engine-routing knobs (sim-tuned; see prof.py sweeps)
CFG = {
    "tr_precast": False,   # cast x/cross/x1 to bf16 before PE transpose
    "wcast_eng": "vector",  # weight fp32->bf16 cast engine
    "anorm_eng": "vector",  # A-row normalize engine
}


def _copy_on(nc, eng, out, in_):
    if eng == "scalar":
        nc.scalar.copy(out=out, in_=in_)
    elif eng == "gpsimd":
        nc.gpsimd.tensor_copy(out=out, in_=in_)
    else:
        nc.vector.tensor_copy(out=out, in_=in_)


def _split_multi_waits(nc):
    """The neuronxcc/walrus path encodes at most one sync-wait per
    instruction (the native NRT path expands multi-waits via a wait table,
    which is unavailable here). Split extra waits onto same-engine NOPs
    inserted immediately before the instruction."""
    n_new = 0
    for fn in nc.m.functions:
        for bb in fn.blocks:
            insts = bb.instructions
            i = 0
            while i < len(insts):
                inst = insts[i]
                si = inst.sync_info
                waits = list(si.on_wait) if si and si.on_wait else []
                if len(waits) > 1:
                    si.on_wait[:] = waits[-1:]
                    for w in waits[:-1]:
                        nop = mybir.InstNoOp(
                            name=f"waitnop_{n_new}",
                            engine=inst.engine,
                            bass_nofuse=True,
                            sync_info=mybir.SyncInfo(on_wait=[w], on_update=[]),
                        )
                        insts.insert(i, nop)
                        n_new += 1
                        i += 1
                i += 1
    return n_new


def _load_weight_bf16(nc, wbf, w_dram, n_in_chunks):
    """DMA a host-precast [n_in_chunks*128, width] bf16 DRAM weight into the
    persistent bf16 SBUF tile ``wbf`` of shape [128, n_in_chunks, width]."""
    w_r = w_dram.ap().rearrange("(c p) f -> p c f", p=P)
    for c in range(n_in_chunks):
        nc.sync.dma_start(wbf[:, c], w_r[:, c])


def _transpose_into(nc, ps_pool, ident_b, dst_bf, src_bf, col_base,
                    copy_eng="vector"):
    """PE-transpose the [128, 8*128] bf16 SBUF tile ``src_bf`` into
    dst_bf[:, dc, col_base:col_base+128]."""
    for dc in range(NDC):
        pt = ps_pool.tile([P, FD], bf16, tag="ps")
        nc.tensor.transpose(pt[:, :P], src_bf[:, dc * P:(dc + 1) * P], ident_b)
        _copy_on(nc, copy_eng, dst_bf[:, dc, col_base:col_base + P], pt[:, :P])


def _layernorm_rows(nc, pool_small, pre, out_tile, eps_tile, gain_bc, bias_bc):
    """LayerNorm along the free dim (1024) of ``pre`` [128, 1024] fp32 into
    ``out_tile``. gain_bc/bias_bc are optional [128, 1024] broadcast tiles."""
    stats = pool_small.tile([P, 2, 6], f32, tag="bnstats")
    for g in range(2):
        nc.vector.bn_stats(out=stats[:, g], in_=pre[:, g * 512:(g + 1) * 512])
    mv = pool_small.tile([P, 2], f32, tag="bnaggr")
    nc.vector.bn_aggr(out=mv[:], in_=stats[:])
    rstd = pool_small.tile([P, 1], f32, tag="rstd")
    nc.scalar.activation(
        out=rstd[:], in_=mv[:, 1:2], func=AF.Sqrt, bias=eps_tile[:], scale=1.0
    )
    nc.vector.reciprocal(out=rstd[:], in_=rstd[:])
    nc.vector.tensor_scalar(
        out=out_tile[:],
        in0=pre[:],
        scalar1=mv[:, 0:1],
        scalar2=rstd[:],
        op0=ALU.subtract,
        op1=ALU.mult,
    )
    if gain_bc is not None:
        nc.vector.tensor_tensor(out_tile[:], out_tile[:], gain_bc[:], ALU.mult)
    if bias_bc is not None:
        nc.vector.tensor_tensor(out_tile[:], out_tile[:], bias_bc[:], ALU.add)


def _bcast_dma(nc, pool, vec_dram, width):
    """Broadcast a [width] DRAM vector across all 128 partitions."""
    t = pool.tile([P, width], f32, tag=f"bcast{width}")
    src = vec_dram.ap()
    bc = bass.AP(tensor=src.tensor, offset=src.offset, ap=[[0, P]] + list(src.ap))
    nc.gpsimd.dma_start(out=t[:], in_=bc)
    return t


def _emit(tc, nc, io, flags):
    x, out_d, attn_d, x1_scr = (
        io["x"], io["out"], io["attn"], io["x1_scr"]
    )
    W = io["W"]          # dict of weight dram handles
    b = io["b"]          # dict of bias/gain dram handles

    x_r = x.ap().rearrange("(t p) d -> t p d", p=P)
    xb_r = io["xb"].ap().rearrange("(t p) d -> t p d", p=P)
    crossb_r = io["crossb"].ap().rearrange("(t p) d -> t p d", p=P)
    out_r = out_d.ap().rearrange("(t p) d -> t p d", p=P)
    attn_r = attn_d.ap().rearrange("h (t p) s -> h t p s", p=P)
    x1_r = x1_scr.ap().rearrange("(t p) d -> t p d", p=P)

    with (
        tc.tile_pool(name="const", bufs=1) as const,
        tc.tile_pool(name="ps", bufs=4, space="PSUM") as ps,
        tc.tile_pool(name="acc_av", bufs=2, space="PSUM") as acc_av,
        tc.tile_pool(name="acc", bufs=2, space="PSUM") as acc,
    ):
        ident_f = const.tile([P, P], f32)
        make_identity(nc, ident_f)
        ident_bf = const.tile([P, P], bf16)
        make_identity(nc, ident_bf)
        idents = (ident_f, ident_bf)
        eps_tile = const.tile([P, 1], f32)
        nc.vector.memset(eps_tile, EPS)
        ones_bf = const.tile([P, 1], bf16)
        nc.vector.memset(ones_bf, 1.0)
        ones_b1 = const.tile([1, P], bf16)
        nc.vector.memset(ones_b1, 1.0)

        # optional bias/gain tiles (skipped when trivial: the common case)
        bq_t = bk_t = b1_t = None
        bv_bc = bo_bc = b2_bc = g1_bc = be1_bc = g2_bc = be2_bc = None
        if flags["bq"]:
            bq_t = const.tile([P, H], f32)
            nc.sync.dma_start(bq_t[:], b["bq"].ap().rearrange("(t p) -> p t", p=P))
        if flags["bk"]:
            bk_t = const.tile([P, H], f32)
            nc.sync.dma_start(bk_t[:], b["bk"].ap().rearrange("(t p) -> p t", p=P))
        if flags["b1"]:
            b1_t = const.tile([P, NFC], f32)
            nc.sync.dma_start(b1_t[:], b["b1"].ap().rearrange("(t p) -> p t", p=P))
        if flags["bv"]:
            bv_bc = _bcast_dma(nc, const, b["bv"], HE)
        if flags["bo"]:
            bo_bc = _bcast_dma(nc, const, b["bo"], D)
        if flags["b2"]:
            b2_bc = _bcast_dma(nc, const, b["b2"], D)
        if flags["g1"]:
            g1_bc = _bcast_dma(nc, const, b["g1"], D)
        if flags["be1"]:
            be1_bc = _bcast_dma(nc, const, b["be1"], D)
        if flags["g2"]:
            g2_bc = _bcast_dma(nc, const, b["g2"], D)
        if flags["be2"]:
            be2_bc = _bcast_dma(nc, const, b["be2"], D)

        with (
            tc.tile_pool(name="small", bufs=4) as small,
            tc.tile_pool(name="x1T", bufs=1) as pool_x1T,
        ):
            x1T_bf = pool_x1T.tile([P, NDC, L], bf16)

            # ---------------- Phase A: transposes + QKV + A-softmax --------
            with tc.tile_pool(name="qkv", bufs=1) as pool_qkv:
                qT_bf = pool_qkv.tile([P, H, L], bf16, tag="qT")
                kT_bf = pool_qkv.tile([P, H, S], bf16, tag="kT")
                v_bf = pool_qkv.tile([P, NST, HE], bf16, tag="v")
                # per-head softmax reciprocal rows broadcast to all partitions
                bc_all = pool_qkv.tile([P, H, L], bf16, tag="bc")
                recA_all = pool_qkv.tile([P, H, NLT], f32, tag="recA")

                with (
                    tc.tile_pool(name="a_stage", bufs=4) as a_stage,
                    tc.tile_pool(name="b_w", bufs=1) as b_w,
                    tc.tile_pool(name="expA", bufs=3) as expA_pool,
                ):
                    wo_bf = b_w.tile([P, H, D], bf16, tag="wo")

                    # -- upfront PE-dense block: transposes, q/k/v projs ----
                    with (
                        tc.tile_pool(name="a_wkv", bufs=1) as a_wkv,
                        tc.tile_pool(name="a_trc", bufs=1) as a_trc,
                    ):
                        wq_bf = a_wkv.tile([P, NDC, HE], bf16, tag="wq")
                        xT_bf = a_trc.tile([P, NDC, L], bf16, tag="xT")
                        wk_bf = a_wkv.tile([P, NDC, HE], bf16, tag="wk")
                        wv_bf = a_wkv.tile([P, NDC, HE], bf16, tag="wv")
                        crossT_bf = a_trc.tile([P, NDC, S], bf16, tag="crossT")
                        for st_i in range(NST):
                            st = a_stage.tile([P, D], bf16, tag="src")
                            nc.sync.dma_start(st[:], crossb_r[st_i])
                            _transpose_into(
                                nc, ps, ident_bf, crossT_bf, st, st_i * P
                            )
                        for lt in range(NLT):
                            st = a_stage.tile([P, D], bf16, tag="src")
                            nc.sync.dma_start(st[:], xb_r[lt])
                            _transpose_into(nc, ps, ident_bf, xT_bf, st, lt * P)
                        _load_weight_bf16(nc, wk_bf, W["Wk"], NDC)
                        _load_weight_bf16(nc, wv_bf, W["Wv"], NDC)
                        _load_weight_bf16(nc, wq_bf, W["Wq"], NDC)

                        def emit_layoutA(h):
                            # layout-A softmax: A rows out + recip columns.
                            # Deferred heads run during the attention loop,
                            # where the projection `acc` pool is idle -- use
                            # it for their score tiles to decouple from the
                            # scoresT/bc traffic on `ps`.
                            sc_pool, sc_tag = (
                                (ps, "ps") if h < H_SPLIT else (acc, "acc")
                            )
                            recA = recA_all[:, h]
                            for lt in range(NLT):
                                ea = expA_pool.tile([P, S], f32, tag="expA")
                                parts = small.tile([P, 2], f32, tag="parts")
                                for nh in range(NH):
                                    pa = sc_pool.tile([P, FD], f32, tag=sc_tag)
                                    nc.tensor.matmul(
                                        pa[:],
                                        qT_bf[:, h, lt * P:(lt + 1) * P],
                                        kT_bf[:, h, nh * FD:(nh + 1) * FD],
                                        start=True,
                                        stop=True,
                                    )
                                    nc.scalar.activation(
                                        out=ea[:, nh * FD:(nh + 1) * FD],
                                        in_=pa[:],
                                        func=AF.Exp,
                                        scale=SCALE,
                                        accum_out=parts[:, nh:nh + 1],
                                    )
                                den = small.tile([P, 1], f32, tag="den")
                                nc.vector.tensor_tensor(
                                    den[:], parts[:, 0:1], parts[:, 1:2],
                                    ALU.add,
                                )
                                nc.vector.reciprocal(
                                    out=recA[:, lt:lt + 1], in_=den[:]
                                )
                                nc.vector.tensor_scalar_mul(
                                    out=ea[:], in0=ea[:],
                                    scalar1=recA[:, lt:lt + 1],
                                )
                                nc.sync.dma_start(attn_r[h, lt], ea[:])

                        for h in range(H):
                            for nh in range(NH):
                                pk = acc.tile([P, FD], f32, tag="acc")
                                for dc in range(NDC):
                                    nc.tensor.matmul(
                                        pk[:],
                                        wk_bf[:, dc, h * E:(h + 1) * E],
                                        crossT_bf[:, dc, nh * FD:(nh + 1) * FD],
                                        start=(dc == 0),
                                        stop=(dc == NDC - 1),
                                    )
                                dst = kT_bf[:, h, nh * FD:(nh + 1) * FD]
                                if bk_t is not None:
                                    nc.vector.tensor_scalar_add(
                                        out=dst, in0=pk[:],
                                        scalar1=bk_t[:, h:h + 1],
                                    )
                                else:
                                    nc.vector.tensor_copy(out=dst, in_=pk[:])
                            for nh in range(NH):
                                pq = acc.tile([P, FD], f32, tag="acc")
                                for dc in range(NDC):
                                    nc.tensor.matmul(
                                        pq[:],
                                        wq_bf[:, dc, h * E:(h + 1) * E],
                                        xT_bf[:, dc, nh * FD:(nh + 1) * FD],
                                        start=(dc == 0),
                                        stop=(dc == NDC - 1),
                                    )
                                dst = qT_bf[:, h, nh * FD:(nh + 1) * FD]
                                if bq_t is not None:
                                    nc.vector.tensor_scalar_add(
                                        out=dst, in0=pq[:],
                                        scalar1=bq_t[:, h:h + 1],
                                    )
                                else:
                                    nc.vector.tensor_copy(out=dst, in_=pq[:])
                            if h < H_SPLIT:
                                emit_layoutA(h)
                            # v[s_t=h] = cross_t @ Wv (natural [S, HE])
                            for nh in range(NH):
                                pv = acc.tile([P, FD], f32, tag="acc")
                                for dc in range(NDC):
                                    nc.tensor.matmul(
                                        pv[:],
                                        crossT_bf[:, dc, h * P:(h + 1) * P],
                                        wv_bf[:, dc, nh * FD:(nh + 1) * FD],
                                        start=(dc == 0),
                                        stop=(dc == NDC - 1),
                                    )
                                dst = v_bf[:, h, nh * FD:(nh + 1) * FD]
                                if bv_bc is not None:
                                    nc.vector.tensor_tensor(
                                        dst, pv[:],
                                        bv_bc[:, nh * FD:(nh + 1) * FD],
                                        ALU.add,
                                    )
                                else:
                                    nc.vector.tensor_copy(out=dst, in_=pv[:])

                    # -- per-head pipeline: qT, scores^T+exp, layout-A, AV --
                    _load_weight_bf16(nc, wo_bf, W["Wo"], H)
                    _outT_cm = tc.tile_pool(name="outT", bufs=1)
                    pool_outT = _outT_cm.__enter__()
                    outT_bf = pool_outT.tile([P, H, L], bf16)
                    with tc.tile_pool(name="expT", bufs=3) as expT_pool:
                        for h in range(H):
                            qTh = qT_bf[:, h]
                            kTh = kT_bf[:, h]
                            if h >= H_SPLIT:
                                emit_layoutA(h)
                            # scores^T = k_h @ q_h^T, exp -> expT (bf16)
                            expT = expT_pool.tile([P, NST, L], bf16, tag="expT")
                            for st_i in range(NST):
                                for nh in range(NH):
                                    pt = ps.tile([P, FD], f32, tag="ps")
                                    nc.tensor.matmul(
                                        pt[:],
                                        kTh[:, st_i * P:(st_i + 1) * P],
                                        qTh[:, nh * FD:(nh + 1) * FD],
                                        start=True,
                                        stop=True,
                                    )
                                    nc.scalar.activation(
                                        out=expT[:, st_i, nh * FD:(nh + 1) * FD],
                                        in_=pt[:],
                                        func=AF.Exp,
                                        scale=SCALE,
                                    )
                            # out^T_h = v_h^T @ expT (accumulate over s)
                            pavs = []
                            for nh in range(NH):
                                pav = acc_av.tile([P, FD], f32, tag="acc_av")
                                for st_i in range(NST):
                                    nc.tensor.matmul(
                                        pav[:],
                                        v_bf[:, st_i, h * E:(h + 1) * E],
                                        expT[:, st_i, nh * FD:(nh + 1) * FD],
                                        start=(st_i == 0),
                                        stop=(st_i == NST - 1),
                                    )
                                pavs.append(pav)
                            # recip columns -> [1, L] row on partition 0, then
                            # K=1 ones-matmuls broadcast across partitions.
                            rr_sb = small.tile([1, L], bf16, tag="rrsb")
                            for lt in range(NLT):
                                ptr = ps.tile([P, FD], f32, tag="ps")
                                nc.tensor.transpose(
                                    ptr[:1, :P], recA_all[:, h, lt:lt + 1],
                                    ident_f,
                                )
                                nc.vector.tensor_copy(
                                    out=rr_sb[:, lt * P:(lt + 1) * P],
                                    in_=ptr[:1, :P],
                                )
                            for nh in range(NH):
                                pb = ps.tile([P, FD], f32, tag="ps")
                                nc.tensor.matmul(
                                    pb[:], ones_b1[:],
                                    rr_sb[:, nh * FD:(nh + 1) * FD],
                                    start=True, stop=True,
                                )
                                nc.vector.tensor_copy(
                                    out=bc_all[:, h, nh * FD:(nh + 1) * FD],
                                    in_=pb[:],
                                )
                            for nh in range(NH):
                                nc.vector.tensor_tensor(
                                    outT_bf[:, h, nh * FD:(nh + 1) * FD],
                                    pavs[nh][:],
                                    bc_all[:, h, nh * FD:(nh + 1) * FD],
                                    ALU.mult,
                                )

                    # ---- out-projection + residual + LN1 + x1^T -------
                    if True:
                        with tc.tile_pool(name="ln1", bufs=2) as ln1:
                            for lt in range(NLT):
                                p0 = acc.tile([P, FD], f32, tag="acc")
                                p1 = acc.tile([P, FD], f32, tag="acc")
                                for hc in range(H):
                                    nc.tensor.matmul(
                                        p0[:],
                                        outT_bf[:, hc, lt * P:(lt + 1) * P],
                                        wo_bf[:, hc, 0:FD],
                                        start=(hc == 0), stop=(hc == H - 1),
                                    )
                                for hc in range(H):
                                    nc.tensor.matmul(
                                        p1[:],
                                        outT_bf[:, hc, lt * P:(lt + 1) * P],
                                        wo_bf[:, hc, FD:2 * FD],
                                        start=(hc == 0), stop=(hc == H - 1),
                                    )
                                xres = ln1.tile([P, D], f32, tag="xres")
                                nc.sync.dma_start(xres[:], x_r[lt])
                                pre = ln1.tile([P, D], f32, tag="pre")
                                nc.vector.tensor_tensor(
                                    pre[:, 0:FD], p0[:], xres[:, 0:FD], ALU.add
                                )
                                nc.vector.tensor_tensor(
                                    pre[:, FD:2 * FD], p1[:], xres[:, FD:2 * FD],
                                    ALU.add,
                                )
                                if bo_bc is not None:
                                    nc.vector.tensor_tensor(
                                        pre[:], pre[:], bo_bc[:], ALU.add
                                    )
                                x1t = ln1.tile([P, D], f32, tag="x1t")
                                _layernorm_rows(
                                    nc, small, pre, x1t, eps_tile, g1_bc, be1_bc
                                )
                                nc.sync.dma_start(x1_r[lt], x1t[:])
                                x1b = ln1.tile([P, D], bf16, tag="x1b")
                                nc.gpsimd.tensor_copy(out=x1b[:], in_=x1t[:])
                                _transpose_into(
                                    nc, ps, ident_bf, x1T_bf, x1b, lt * P,
                                    copy_eng="scalar",
                                )
                    _outT_cm.__exit__(None, None, None)

            # ---------------- Phase C: conv1x1 FFN + LN2 -------------------
            with (
                tc.tile_pool(name="c_w2", bufs=1) as c_w2,
                tc.tile_pool(name="c_hT", bufs=1) as c_hT,
                tc.tile_pool(name="c_w1", bufs=3) as c_w1,
                tc.tile_pool(name="c_ln", bufs=2) as c_ln,
            ):
                w2_bf = c_w2.tile([P, NFC, D], bf16, tag="w2")
                w2_r = W["W2"].ap().rearrange("(c p) f -> p c f", p=P)
                hT_bf = c_hT.tile([P, NFC, L], bf16)

                # stream W1 in dff-column blocks of G, loaded as natural row
                # chunks (contiguous >=1KB runs per partition)
                G = 512
                w1_ap = W["W1"].ap()
                for fc in range(NFC):
                    # one W2 row-chunk per fc iteration (overlaps hT compute)
                    nc.sync.dma_start(w2_bf[:, fc], w2_r[:, fc])
                    if fc % (G // P) == 0:
                        g0 = fc * P
                        w1b = c_w1.tile([P, NDC, G], bf16, tag="w1b")
                        for dc in range(NDC):
                            nc.sync.dma_start(
                                w1b[:, dc],
                                w1_ap[dc * P:(dc + 1) * P, g0:g0 + G],
                            )
                    fl = (fc % (G // P)) * P
                    for nh in range(NH):
                        ph = acc.tile([P, FD], f32, tag="acc")
                        for dc in range(NDC):
                            nc.tensor.matmul(
                                ph[:],
                                w1b[:, dc, fl:fl + P],
                                x1T_bf[:, dc, nh * FD:(nh + 1) * FD],
                                start=(dc == 0),
                                stop=(dc == NDC - 1),
                            )
                        if b1_t is not None:
                            nc.vector.tensor_scalar(
                                out=hT_bf[:, fc, nh * FD:(nh + 1) * FD],
                                in0=ph[:],
                                scalar1=b1_t[:, fc:fc + 1],
                                scalar2=0.0,
                                op0=ALU.add,
                                op1=ALU.max,
                            )
                        else:
                            nc.vector.tensor_scalar_max(
                                out=hT_bf[:, fc, nh * FD:(nh + 1) * FD],
                                in0=ph[:],
                                scalar1=0.0,
                            )

                for lt in range(NLT):
                    p0 = acc.tile([P, FD], f32, tag="acc")
                    p1 = acc.tile([P, FD], f32, tag="acc")
                    for fc in range(NFC):
                        nc.tensor.matmul(
                            p0[:],
                            hT_bf[:, fc, lt * P:(lt + 1) * P],
                            w2_bf[:, fc, 0:FD],
                            start=(fc == 0), stop=(fc == NFC - 1),
                        )
                    for fc in range(NFC):
                        nc.tensor.matmul(
                            p1[:],
                            hT_bf[:, fc, lt * P:(lt + 1) * P],
                            w2_bf[:, fc, FD:2 * FD],
                            start=(fc == 0), stop=(fc == NFC - 1),
                        )
                    x1res = c_ln.tile([P, D], f32, tag="x1res")
                    nc.sync.dma_start(x1res[:], x1_r[lt])
                    pre = c_ln.tile([P, D], f32, tag="pre2")
                    nc.vector.tensor_tensor(
                        pre[:, 0:FD], p0[:], x1res[:, 0:FD], ALU.add
                    )
                    nc.vector.tensor_tensor(
                        pre[:, FD:2 * FD], p1[:], x1res[:, FD:2 * FD], ALU.add
                    )
                    if b2_bc is not None:
                        nc.vector.tensor_tensor(pre[:], pre[:], b2_bc[:], ALU.add)
                    _layernorm_rows(nc, small, pre, pre, eps_tile, g2_bc, be2_bc)
                    nc.sync.dma_start(out_r[lt], pre[:])


def _build(flags):
    _patch_tile_drain()
    nc = bass.Bass("TRN2", target_bir_lowering=False, debug=False, num_devices=8)
    io = {}
    io["x"] = nc.declare_dram_parameter("x", [L, D], f32, isOutput=False)
    io["xb"] = nc.declare_dram_parameter("xb", [L, D], bf16, isOutput=False)
    io["crossb"] = nc.declare_dram_parameter(
        "crossb", [S, D], bf16, isOutput=False
    )
    W = {}
    W["Wq"] = nc.declare_dram_parameter("Wq", [D, HE], bf16, isOutput=False)
    W["Wk"] = nc.declare_dram_parameter("Wk", [D, HE], bf16, isOutput=False)
    W["Wv"] = nc.declare_dram_parameter("Wv", [D, HE], bf16, isOutput=False)
    W["Wo"] = nc.declare_dram_parameter("Wo", [HE, D], bf16, isOutput=False)
    W["W1"] = nc.declare_dram_parameter("W1", [D, DFF], bf16, isOutput=False)
    W["W2"] = nc.declare_dram_parameter("W2", [DFF, D], bf16, isOutput=False)
    io["W"] = W
    b = {}
    for name, dim in [
        ("bq", HE), ("bk", HE), ("bv", HE), ("bo", D),
        ("b1", DFF), ("b2", D), ("g1", D), ("be1", D), ("g2", D), ("be2", D),
    ]:
        if flags[name]:
            b[name] = nc.declare_dram_parameter(name, [dim], f32, isOutput=False)
    io["b"] = b
    io["out"] = nc.declare_dram_parameter("out", [L, D], f32, isOutput=True)
    io["attn"] = nc.declare_dram_parameter("attn", [H, L, S], f32, isOutput=True)
    io["x1_scr"] = nc.dram_tensor("x1_scr", [L, D], f32)

    with tile.TileContext(nc) as tc:
        _emit(tc, nc, io, flags)
    _split_multi_waits(nc)
    return nc


_CACHE = {}


def _get_nc(flags):
    key = tuple(sorted(flags.items()))
    if key not in _CACHE:
        _CACHE[key] = _build(flags)
    return _CACHE[key]


def kernel(**inputs):
    B = 8
    flags = {
        "bq": bool(np.any(inputs["bq"])),
        "bk": bool(np.any(inputs["bk"])),
        "bv": bool(np.any(inputs["bv"])),
        "bo": bool(np.any(inputs["bo"])),
        "b1": bool(np.any(inputs["b1"])),
        "b2": bool(np.any(inputs["b2"])),
        "g1": bool(np.any(np.asarray(inputs["g1"]) != 1.0)),
        "be1": bool(np.any(inputs["be1"])),
        "g2": bool(np.any(np.asarray(inputs["g2"]) != 1.0)),
        "be2": bool(np.any(inputs["be2"])),
    }
    nc = _get_nc(flags)

    import ml_dtypes

    bf = ml_dtypes.bfloat16
    shared = {}
    for name in ["Wq", "Wk", "Wv", "Wo", "W1", "W2"]:
        shared[name] = np.ascontiguousarray(
            np.asarray(inputs[name], np.float32).astype(bf)
        )
    for name, on in flags.items():
        if on:
            shared[name] = np.ascontiguousarray(np.asarray(inputs[name], np.float32))

    in_maps = []
    x_full = np.asarray(inputs["x"], np.float32)
    c_full = np.asarray(inputs["cross"], np.float32)
    for core in range(B):
        m = dict(shared)
        m["x"] = np.ascontiguousarray(x_full[core])
        m["xb"] = np.ascontiguousarray(x_full[core].astype(bf))
        m["crossb"] = np.ascontiguousarray(c_full[core].astype(bf))
        in_maps.append(m)

    res = run_bass_kernel_spmd(nc, in_maps, list(range(B)))
    out = np.stack([res.results[i]["out"] for i in range(B)], axis=0)
    A = np.stack([res.results[i]["attn"] for i in range(B)], axis=0)
    return out, A


# revision 122
# speedup vs baseline: 1.0012x; 1.0012x over previous
"""Trainium2 Bass kernel for nn_AttentionBlock (cross-attention block + FFN).

Sharding: data-parallel over batch B=8 across the 8 NeuronCores (one batch
element per core). Each core computes the full attention block for its batch
element:
    q = x@Wq+bq, k = cross@Wk+bk, v = cross@Wv+bv        (per-head E=128)
    A = softmax(q k^T / sqrt(E)); out = A v
    x1 = LN(out@Wo + bo + x)
    y  = relu(x1@W1+b1)@W2 + b2
    ret = LN(x1 + y), A

All matmuls run in bf16 on the PE array with fp32 PSUM accumulation;
softmax/layernorm statistics and both outputs stay fp32.
"""

import sys

sys.path.insert(0, "/opt/trn_rl_repo")

import numpy as np

import concourse.bass as bass
import concourse.mybir as mybir
import concourse.tile as tile
from concourse.bass_utils import run_bass_kernel_spmd
from concourse.masks import make_identity
from concourse.tile import ScopedClock

P = 128
L = 1024
S = 1024
D = 1024
H = 8
E = 128
HE = 1024
DFF = 4096
NLT = L // P     # 8 l-tiles
NST = S // P     # 8 s-tiles
NDC = D // P     # 8 d-chunks
NFC = DFF // P   # 32 ff-chunks
NH = 2           # 512-wide halves of a 1024 free dim
FD = 512         # matmul free dim (one PSUM bank of fp32)
H_SPLIT = 6      # layout-A heads computed in the upfront block
SCALE = float(1.0 / np.sqrt(E))
EPS = 1e-5

f32 = mybir.dt.float32
bf16 = mybir.dt.bfloat16
AF = mybir.ActivationFunctionType
ALU = mybir.AluOpType


def _patch_tile_drain():
    """This walrus build rejects >1 sync-wait on the CTRL Drain instruction
    emitted at TileContext exit; split the waits across several drains."""
    if getattr(tile.TileContext, "_drain_patched", False):
        return

    def _drain_and_barrier(self, tick_clock, wait_clock):
        drain_inst = self.nc.sync.drain()
        wait_clock.add_sem_waits(
            drain_inst.ins, ScopedClock({None: tick_clock.global_clock})
        )
        si = drain_inst.ins.sync_info
        waits = list(si.on_wait) if si and si.on_wait else []
        if len(waits) > 1:
            si.on_wait[:] = waits[:1]
            for w in waits[1:]:
                d2 = self.nc.sync.drain()
                si2 = d2.ins.sync_info
                if si2 is None:
                    d2.ins.sync_info = mybir.SyncInfo(on_wait=[w], on_update=[])
                else:
                    si2.on_wait[:] = [w]
        self.nc.all_engine_barrier()
        assert self.sems is not None
        popped = self.nc._tile_sem_poison_stack.pop()
        assert popped is self._sem_poison
        self.nc.clear_and_free_semaphores(list(self.sems.allocated().values()))
        self.nc.all_engine_barrier()

    tile.TileContext._drain_and_barrier = _drain_and_barrier
    tile.TileContext._drain_patched = True


# engine-routing knobs (sim-tuned; see prof.py sweeps)
CFG = {
    "tr_precast": False,   # cast x/cross/x1 to bf16 before PE transpose
    "wcast_eng": "vector",  # weight fp32->bf16 cast engine
    "anorm_eng": "vector",  # A-row normalize engine
}


def _copy_on(nc, eng, out, in_):
    if eng == "scalar":
        nc.scalar.copy(out=out, in_=in_)
    elif eng == "gpsimd":
        nc.gpsimd.tensor_copy(out=out, in_=in_)
    else:
        nc.vector.tensor_copy(out=out, in_=in_)


def _split_multi_waits(nc):
    """The neuronxcc/walrus path encodes at most one sync-wait per
    instruction (the native NRT path expands multi-waits via a wait table,
    which is unavailable here). Split extra waits onto same-engine NOPs
    inserted immediately before the instruction."""
    n_new = 0
    for fn in nc.m.functions:
        for bb in fn.blocks:
            insts = bb.instructions
            i = 0
            while i < len(insts):
                inst = insts[i]
                si = inst.sync_info
                waits = list(si.on_wait) if si and si.on_wait else []
                if len(waits) > 1:
                    si.on_wait[:] = waits[-1:]
                    for w in waits[:-1]:
                        nop = mybir.InstNoOp(
                            name=f"waitnop_{n_new}",
                            engine=inst.engine,
                            bass_nofuse=True,
                            sync_info=mybir.SyncInfo(on_wait=[w], on_update=[]),
                        )
                        insts.insert(i, nop)
                        n_new += 1
                        i += 1
                i += 1
    return n_new


def _load_weight_bf16(nc, wbf, w_dram, n_in_chunks):
    """DMA a host-precast [n_in_chunks*128, width] bf16 DRAM weight into the
    persistent bf16 SBUF tile ``wbf`` of shape [128, n_in_chunks, width]."""
    w_r = w_dram.ap().rearrange("(c p) f -> p c f", p=P)
    for c in range(n_in_chunks):
        nc.sync.dma_start(wbf[:, c], w_r[:, c])


def _transpose_into(nc, ps_pool, ident_b, dst_bf, src_bf, col_base,
                    copy_eng="vector"):
    """PE-transpose the [128, 8*128] bf16 SBUF tile ``src_bf`` into
    dst_bf[:, dc, col_base:col_base+128]."""
    for dc in range(NDC):
        pt = ps_pool.tile([P, FD], bf16, tag="ps")
        nc.tensor.transpose(pt[:, :P], src_bf[:, dc * P:(dc + 1) * P], ident_b)
        _copy_on(nc, copy_eng, dst_bf[:, dc, col_base:col_base + P], pt[:, :P])


def _layernorm_rows(nc, pool_small, pre, out_tile, eps_tile, gain_bc, bias_bc):
    """LayerNorm along the free dim (1024) of ``pre`` [128, 1024] fp32 into
    ``out_tile``. gain_bc/bias_bc are optional [128, 1024] broadcast tiles."""
    stats = pool_small.tile([P, 2, 6], f32, tag="bnstats")
    for g in range(2):
        nc.vector.bn_stats(out=stats[:, g], in_=pre[:, g * 512:(g + 1) * 512])
    mv = pool_small.tile([P, 2], f32, tag="bnaggr")
    nc.vector.bn_aggr(out=mv[:], in_=stats[:])
    rstd = pool_small.tile([P, 1], f32, tag="rstd")
    nc.scalar.activation(
        out=rstd[:], in_=mv[:, 1:2], func=AF.Sqrt, bias=eps_tile[:], scale=1.0
    )
    nc.vector.reciprocal(out=rstd[:], in_=rstd[:])
    nc.vector.tensor_scalar(
        out=out_tile[:],
        in0=pre[:],
        scalar1=mv[:, 0:1],
        scalar2=rstd[:],
        op0=ALU.subtract,
        op1=ALU.mult,
    )
    if gain_bc is not None:
        nc.vector.tensor_tensor(out_tile[:], out_tile[:], gain_bc[:], ALU.mult)
    if bias_bc is not None:
        nc.vector.tensor_tensor(out_tile[:], out_tile[:], bias_bc[:], ALU.add)


def _bcast_dma(nc, pool, vec_dram, width):
    """Broadcast a [width] DRAM vector across all 128 partitions."""
    t = pool.tile([P, width], f32, tag=f"bcast{width}")
    src = vec_dram.ap()
    bc = bass.AP(tensor=src.tensor, offset=src.offset, ap=[[0, P]] + list(src.ap))
    nc.gpsimd.dma_start(out=t[:], in_=bc)
    return t


def _emit(tc, nc, io, flags):
    x, out_d, attn_d, x1_scr = (
        io["x"], io["out"], io["attn"], io["x1_scr"]
    )
    W = io["W"]          # dict of weight dram handles
    b = io["b"]          # dict of bias/gain dram handles

    x_r = x.ap().rearrange("(t p) d -> t p d", p=P)
    xb_r = io["xb"].ap().rearrange("(t p) d -> t p d", p=P)
    crossb_r = io["crossb"].ap().rearrange("(t p) d -> t p d", p=P)
    out_r = out_d.ap().rearrange("(t p) d -> t p d", p=P)
    attn_r = attn_d.ap().rearrange("h (t p) s -> h t p s", p=P)
    x1_r = x1_scr.ap().rearrange("(t p) d -> t p d", p=P)

    with (
        tc.tile_pool(name="const", bufs=1) as const,
        tc.tile_pool(name="ps", bufs=4, space="PSUM") as ps,
        tc.tile_pool(name="acc_av", bufs=2, space="PSUM") as acc_av,
        tc.tile_pool(name="acc", bufs=2, space="PSUM") as acc,
    ):
        ident_f = const.tile([P, P], f32)
        make_identity(nc, ident_f)
        ident_bf = const.tile([P, P], bf16)
        make_identity(nc, ident_bf)
        idents = (ident_f, ident_bf)
        eps_tile = const.tile([P, 1], f32)
        nc.vector.memset(eps_tile, EPS)
        ones_bf = const.tile([P, 1], bf16)
        nc.vector.memset(ones_bf, 1.0)
        ones_b1 = const.tile([1, P], bf16)
        nc.vector.memset(ones_b1, 1.0)

        # optional bias/gain tiles (skipped when trivial: the common case)
        bq_t = bk_t = b1_t = None
        bv_bc = bo_bc = b2_bc = g1_bc = be1_bc = g2_bc = be2_bc = None
        if flags["bq"]:
            bq_t = const.tile([P, H], f32)
            nc.sync.dma_start(bq_t[:], b["bq"].ap().rearrange("(t p) -> p t", p=P))
        if flags["bk"]:
            bk_t = const.tile([P, H], f32)
            nc.sync.dma_start(bk_t[:], b["bk"].ap().rearrange("(t p) -> p t", p=P))
        if flags["b1"]:
            b1_t = const.tile([P, NFC], f32)
            nc.sync.dma_start(b1_t[:], b["b1"].ap().rearrange("(t p) -> p t", p=P))
        if flags["bv"]:
            bv_bc = _bcast_dma(nc, const, b["bv"], HE)
        if flags["bo"]:
            bo_bc = _bcast_dma(nc, const, b["bo"], D)
        if flags["b2"]:
            b2_bc = _bcast_dma(nc, const, b["b2"], D)
        if flags["g1"]:
            g1_bc = _bcast_dma(nc, const, b["g1"], D)
        if flags["be1"]:
            be1_bc = _bcast_dma(nc, const, b["be1"], D)
        if flags["g2"]:
            g2_bc = _bcast_dma(nc, const, b["g2"], D)
        if flags["be2"]:
            be2_bc = _bcast_dma(nc, const, b["be2"], D)

        with (
            tc.tile_pool(name="small", bufs=4) as small,
            tc.tile_pool(name="x1T", bufs=1) as pool_x1T,
        ):
            x1T_bf = pool_x1T.tile([P, NDC, L], bf16)

            # ---------------- Phase A: transposes + QKV + A-softmax --------
            with tc.tile_pool(name="qkv", bufs=1) as pool_qkv:
                qT_bf = pool_qkv.tile([P, H, L], bf16, tag="qT")
                kT_bf = pool_qkv.tile([P, H, S], bf16, tag="kT")
                v_bf = pool_qkv.tile([P, NST, HE], bf16, tag="v")
                # per-head softmax reciprocal rows broadcast to all partitions
                bc_all = pool_qkv.tile([P, H, L], bf16, tag="bc")
                recA_all = pool_qkv.tile([P, H, NLT], f32, tag="recA")

                with (
                    tc.tile_pool(name="a_stage", bufs=4) as a_stage,
                    tc.tile_pool(name="b_w", bufs=1) as b_w,
                    tc.tile_pool(name="expA", bufs=3) as expA_pool,
                ):
                    wo_bf = b_w.tile([P, H, D], bf16, tag="wo")

                    # -- upfront PE-dense block: transposes, q/k/v projs ----
                    with (
                        tc.tile_pool(name="a_wkv", bufs=1) as a_wkv,
                        tc.tile_pool(name="a_trc", bufs=1) as a_trc,
                    ):
                        wq_bf = a_wkv.tile([P, NDC, HE], bf16, tag="wq")
                        xT_bf = a_trc.tile([P, NDC, L], bf16, tag="xT")
                        wk_bf = a_wkv.tile([P, NDC, HE], bf16, tag="wk")
                        wv_bf = a_wkv.tile([P, NDC, HE], bf16, tag="wv")
                        crossT_bf = a_trc.tile([P, NDC, S], bf16, tag="crossT")
                        for st_i in range(NST):
                            st = a_stage.tile([P, D], bf16, tag="src")
                            nc.sync.dma_start(st[:], crossb_r[st_i])
                            _transpose_into(
                                nc, ps, ident_bf, crossT_bf, st, st_i * P
                            )
                        for lt in range(NLT):
                            st = a_stage.tile([P, D], bf16, tag="src")
                            nc.sync.dma_start(st[:], xb_r[lt])
                            _transpose_into(nc, ps, ident_bf, xT_bf, st, lt * P)
                        _load_weight_bf16(nc, wk_bf, W["Wk"], NDC)
                        _load_weight_bf16(nc, wv_bf, W["Wv"], NDC)
                        _load_weight_bf16(nc, wq_bf, W["Wq"], NDC)

                        def emit_layoutA(h):
                            # layout-A softmax: A rows out + recip columns.
                            # Deferred heads run during the attention loop,
                            # where the projection `acc` pool is idle -- use
                            # it for their score tiles to decouple from the
                            # scoresT/bc traffic on `ps`.
                            sc_pool, sc_tag = (
                                (ps, "ps") if h < H_SPLIT else (acc, "acc")
                            )
                            recA = recA_all[:, h]
                            for lt in range(NLT):
                                ea = expA_pool.tile([P, S], f32, tag="expA")
                                parts = small.tile([P, 2], f32, tag="parts")
                                for nh in range(NH):
                                    pa = sc_pool.tile([P, FD], f32, tag=sc_tag)
                                    nc.tensor.matmul(
                                        pa[:],
                                        qT_bf[:, h, lt * P:(lt + 1) * P],
                                        kT_bf[:, h, nh * FD:(nh + 1) * FD],
                                        start=True,
                                        stop=True,
                                    )
                                    nc.scalar.activation(
                                        out=ea[:, nh * FD:(nh + 1) * FD],
                                        in_=pa[:],
                                        func=AF.Exp,
                                        scale=SCALE,
                                        accum_out=parts[:, nh:nh + 1],
                                    )
                                den = small.tile([P, 1], f32, tag="den")
                                nc.vector.tensor_tensor(
                                    den[:], parts[:, 0:1], parts[:, 1:2],
                                    ALU.add,
                                )
                                nc.vector.reciprocal(
                                    out=recA[:, lt:lt + 1], in_=den[:]
                                )
                                nc.vector.tensor_scalar_mul(
                                    out=ea[:], in0=ea[:],
                                    scalar1=recA[:, lt:lt + 1],
                                )
                                nc.sync.dma_start(attn_r[h, lt], ea[:])

                        for h in range(H):
                            for nh in range(NH):
                                pk = acc.tile([P, FD], f32, tag="acc")
                                for dc in range(NDC):
                                    nc.tensor.matmul(
                                        pk[:],
                                        wk_bf[:, dc, h * E:(h + 1) * E],
                                        crossT_bf[:, dc, nh * FD:(nh + 1) * FD],
                                        start=(dc == 0),
                                        stop=(dc == NDC - 1),
                                    )
                                dst = kT_bf[:, h, nh * FD:(nh + 1) * FD]
                                if bk_t is not None:
                                    nc.vector.tensor_scalar_add(
                                        out=dst, in0=pk[:],
                                        scalar1=bk_t[:, h:h + 1],
                                    )
                                else:
                                    nc.vector.tensor_copy(out=dst, in_=pk[:])
                            for nh in range(NH):
                                pq = acc.tile([P, FD], f32, tag="acc")
                                for dc in range(NDC):
                                    nc.tensor.matmul(
                                        pq[:],
                                        wq_bf[:, dc, h * E:(h + 1) * E],
                                        xT_bf[:, dc, nh * FD:(nh + 1) * FD],
                                        start=(dc == 0),
                                        stop=(dc == NDC - 1),
                                    )
                                dst = qT_bf[:, h, nh * FD:(nh + 1) * FD]
                                if bq_t is not None:
                                    nc.vector.tensor_scalar_add(
                                        out=dst, in0=pq[:],
                                        scalar1=bq_t[:, h:h + 1],
                                    )
                                else:
                                    nc.vector.tensor_copy(out=dst, in_=pq[:])
                            if h < H_SPLIT:
                                emit_layoutA(h)
                            # v[s_t=h] = cross_t @ Wv (natural [S, HE])
                            for nh in range(NH):
                                pv = acc.tile([P, FD], f32, tag="acc")
                                for dc in range(NDC):
                                    nc.tensor.matmul(
                                        pv[:],
                                        crossT_bf[:, dc, h * P:(h + 1) * P],
                                        wv_bf[:, dc, nh * FD:(nh + 1) * FD],
                                        start=(dc == 0),
                                        stop=(dc == NDC - 1),
                                    )
                                dst = v_bf[:, h, nh * FD:(nh + 1) * FD]
                                if bv_bc is not None:
                                    nc.vector.tensor_tensor(
                                        dst, pv[:],
                                        bv_bc[:, nh * FD:(nh + 1) * FD],
                                        ALU.add,
                                    )
                                else:
                                    nc.vector.tensor_copy(out=dst, in_=pv[:])

                    # -- per-head pipeline: qT, scores^T+exp, layout-A, AV --
                    _load_weight_bf16(nc, wo_bf, W["Wo"], H)
                    _outT_cm = tc.tile_pool(name="outT", bufs=1)
                    pool_outT = _outT_cm.__enter__()
                    outT_bf = pool_outT.tile([P, H, L], bf16)
                    with tc.tile_pool(name="expT", bufs=3) as expT_pool:
                        for h in range(H):
                            qTh = qT_bf[:, h]
                            kTh = kT_bf[:, h]
                            if h >= H_SPLIT:
                                emit_layoutA(h)
                            # scores^T = k_h @ q_h^T, exp -> expT (bf16)
                            expT = expT_pool.tile([P, NST, L], bf16, tag="expT")
                            for st_i in range(NST):
                                for nh in range(NH):
                                    pt = ps.tile([P, FD], f32, tag="ps")
                                    nc.tensor.matmul(
                                        pt[:],
                                        kTh[:, st_i * P:(st_i + 1) * P],
                                        qTh[:, nh * FD:(nh + 1) * FD],
                                        start=True,
                                        stop=True,
                                    )
                                    nc.scalar.activation(
                                        out=expT[:, st_i, nh * FD:(nh + 1) * FD],
                                        in_=pt[:],
                                        func=AF.Exp,
                                        scale=SCALE,
                                    )
                            # out^T_h = v_h^T @ expT (accumulate over s)
                            pavs = []
                            for nh in range(NH):
                                pav = acc_av.tile([P, FD], f32, tag="acc_av")
                                for st_i in range(NST):
                                    nc.tensor.matmul(
                                        pav[:],
                                        v_bf[:, st_i, h * E:(h + 1) * E],
                                        expT[:, st_i, nh * FD:(nh + 1) * FD],
                                        start=(st_i == 0),
                                        stop=(st_i == NST - 1),
                                    )
                                pavs.append(pav)
                            # recip columns -> [1, L] row on partition 0, then
                            # K=1 ones-matmuls broadcast across partitions.
                            rr_sb = small.tile([1, L], bf16, tag="rrsb")
                            for lt in range(NLT):
                                ptr = ps.tile([P, FD], f32, tag="ps")
                                nc.tensor.transpose(
                                    ptr[:1, :P], recA_all[:, h, lt:lt + 1],
                                    ident_f,
                                )
                                nc.vector.tensor_copy(
                                    out=rr_sb[:, lt * P:(lt + 1) * P],
                                    in_=ptr[:1, :P],
                                )
                            for nh in range(NH):
                                pb = ps.tile([P, FD], f32, tag="ps")
                                nc.tensor.matmul(
                                    pb[:], ones_b1[:],
                                    rr_sb[:, nh * FD:(nh + 1) * FD],
                                    start=True, stop=True,
                                )
                                nc.vector.tensor_copy(
                                    out=bc_all[:, h, nh * FD:(nh + 1) * FD],
                                    in_=pb[:],
                                )
                            for nh in range(NH):
                                nc.vector.tensor_tensor(
                                    outT_bf[:, h, nh * FD:(nh + 1) * FD],
                                    pavs[nh][:],
                                    bc_all[:, h, nh * FD:(nh + 1) * FD],
                                    ALU.mult,
                                )

                    # ---- out-projection + residual + LN1 + x1^T -------
                    if True:
                        with tc.tile_pool(name="ln1", bufs=3) as ln1:
                            for lt in range(NLT):
                                p0 = acc.tile([P, FD], f32, tag="acc")
                                p1 = acc.tile([P, FD], f32, tag="acc")
                                for hc in range(H):
                                    nc.tensor.matmul(
                                        p0[:],
                                        outT_bf[:, hc, lt * P:(lt + 1) * P],
                                        wo_bf[:, hc, 0:FD],
                                        start=(hc == 0), stop=(hc == H - 1),
                                    )
                                for hc in range(H):
                                    nc.tensor.matmul(
                                        p1[:],
                                        outT_bf[:, hc, lt * P:(lt + 1) * P],
                                        wo_bf[:, hc, FD:2 * FD],
                                        start=(hc == 0), stop=(hc == H - 1),
                                    )
                                xres = ln1.tile([P, D], f32, tag="xres")
                                nc.sync.dma_start(xres[:], x_r[lt])
                                pre = ln1.tile([P, D], f32, tag="pre")
                                nc.vector.tensor_tensor(
                                    pre[:, 0:FD], p0[:], xres[:, 0:FD], ALU.add
                                )
                                nc.vector.tensor_tensor(
                                    pre[:, FD:2 * FD], p1[:], xres[:, FD:2 * FD],
                                    ALU.add,
                                )
                                if bo_bc is not None:
                                    nc.vector.tensor_tensor(
                                        pre[:], pre[:], bo_bc[:], ALU.add
                                    )
                                x1t = ln1.tile([P, D], f32, tag="x1t")
                                _layernorm_rows(
                                    nc, small, pre, x1t, eps_tile, g1_bc, be1_bc
                                )
                                nc.sync.dma_start(x1_r[lt], x1t[:])
                                x1b = ln1.tile([P, D], bf16, tag="x1b")
                                nc.gpsimd.tensor_copy(out=x1b[:], in_=x1t[:])
                                _transpose_into(
                                    nc, ps, ident_bf, x1T_bf, x1b, lt * P,
                                    copy_eng="scalar",
                                )
                    _outT_cm.__exit__(None, None, None)

            # ---------------- Phase C: conv1x1 FFN + LN2 -------------------
            with (
                tc.tile_pool(name="c_w2", bufs=1) as c_w2,
                tc.tile_pool(name="c_hT", bufs=1) as c_hT,
                tc.tile_pool(name="c_w1", bufs=3) as c_w1,
                tc.tile_pool(name="c_ln", bufs=3) as c_ln,
            ):
                w2_bf = c_w2.tile([P, NFC, D], bf16, tag="w2")
                w2_r = W["W2"].ap().rearrange("(c p) f -> p c f", p=P)
                hT_bf = c_hT.tile([P, NFC, L], bf16)

                # stream W1 in dff-column blocks of G, loaded as natural row
                # chunks (contiguous >=1KB runs per partition)
                G = 512
                w1_ap = W["W1"].ap()
                for fc in range(NFC):
                    # one W2 row-chunk per fc iteration (overlaps hT compute)
                    nc.sync.dma_start(w2_bf[:, fc], w2_r[:, fc])
                    if fc % (G // P) == 0:
                        g0 = fc * P
                        w1b = c_w1.tile([P, NDC, G], bf16, tag="w1b")
                        for dc in range(NDC):
                            nc.sync.dma_start(
                                w1b[:, dc],
                                w1_ap[dc * P:(dc + 1) * P, g0:g0 + G],
                            )
                    fl = (fc % (G // P)) * P
                    for nh in range(NH):
                        ph = acc.tile([P, FD], f32, tag="acc")
                        for dc in range(NDC):
                            nc.tensor.matmul(
                                ph[:],
                                w1b[:, dc, fl:fl + P],
                                x1T_bf[:, dc, nh * FD:(nh + 1) * FD],
                                start=(dc == 0),
                                stop=(dc == NDC - 1),
                            )
                        if b1_t is not None:
                            nc.vector.tensor_scalar(
                                out=hT_bf[:, fc, nh * FD:(nh + 1) * FD],
                                in0=ph[:],
                                scalar1=b1_t[:, fc:fc + 1],
                                scalar2=0.0,
                                op0=ALU.add,
                                op1=ALU.max,
                            )
                        else:
                            nc.vector.tensor_scalar_max(
                                out=hT_bf[:, fc, nh * FD:(nh + 1) * FD],
                                in0=ph[:],
                                scalar1=0.0,
                            )

                for lt in range(NLT):
                    p0 = acc.tile([P, FD], f32, tag="acc")
                    p1 = acc.tile([P, FD], f32, tag="acc")
                    for fc in range(NFC):
                        nc.tensor.matmul(
                            p0[:],
                            hT_bf[:, fc, lt * P:(lt + 1) * P],
                            w2_bf[:, fc, 0:FD],
                            start=(fc == 0), stop=(fc == NFC - 1),
                        )
                    for fc in range(NFC):
                        nc.tensor.matmul(
                            p1[:],
                            hT_bf[:, fc, lt * P:(lt + 1) * P],
                            w2_bf[:, fc, FD:2 * FD],
                            start=(fc == 0), stop=(fc == NFC - 1),
                        )
                    x1res = c_ln.tile([P, D], f32, tag="x1res")
                    nc.sync.dma_start(x1res[:], x1_r[lt])
                    pre = c_ln.tile([P, D], f32, tag="pre2")
                    nc.vector.tensor_tensor(
                        pre[:, 0:FD], p0[:], x1res[:, 0:FD], ALU.add
                    )
                    nc.vector.tensor_tensor(
                        pre[:, FD:2 * FD], p1[:], x1res[:, FD:2 * FD], ALU.add
                    )
                    if b2_bc is not None:
                        nc.vector.tensor_tensor(pre[:], pre[:], b2_bc[:], ALU.add)
                    _layernorm_rows(nc, small, pre, pre, eps_tile, g2_bc, be2_bc)
                    nc.sync.dma_start(out_r[lt], pre[:])


def _build(flags):
    _patch_tile_drain()
    nc = bass.Bass("TRN2", target_bir_lowering=False, debug=False, num_devices=8)
    io = {}
    io["x"] = nc.declare_dram_parameter("x", [L, D], f32, isOutput=False)
    io["xb"] = nc.declare_dram_parameter("xb", [L, D], bf16, isOutput=False)
    io["crossb"] = nc.declare_dram_parameter(
        "crossb", [S, D], bf16, isOutput=False
    )
    W = {}
    W["Wq"] = nc.declare_dram_parameter("Wq", [D, HE], bf16, isOutput=False)
    W["Wk"] = nc.declare_dram_parameter("Wk", [D, HE], bf16, isOutput=False)
    W["Wv"] = nc.declare_dram_parameter("Wv", [D, HE], bf16, isOutput=False)
    W["Wo"] = nc.declare_dram_parameter("Wo", [HE, D], bf16, isOutput=False)
    W["W1"] = nc.declare_dram_parameter("W1", [D, DFF], bf16, isOutput=False)
    W["W2"] = nc.declare_dram_parameter("W2", [DFF, D], bf16, isOutput=False)
    io["W"] = W
    b = {}
    for name, dim in [
        ("bq", HE), ("bk", HE), ("bv", HE), ("bo", D),
        ("b1", DFF), ("b2", D), ("g1", D), ("be1", D), ("g2", D), ("be2", D),
    ]:
        if flags[name]:
            b[name] = nc.declare_dram_parameter(name, [dim], f32, isOutput=False)
    io["b"] = b
    io["out"] = nc.declare_dram_parameter("out", [L, D], f32, isOutput=True)
    io["attn"] = nc.declare_dram_parameter("attn", [H, L, S], f32, isOutput=True)
    io["x1_scr"] = nc.dram_tensor("x1_scr", [L, D], f32)

    with tile.TileContext(nc) as tc:
        _emit(tc, nc, io, flags)
    _split_multi_waits(nc)
    return nc


_CACHE = {}


def _get_nc(flags):
    key = tuple(sorted(flags.items()))
    if key not in _CACHE:
        _CACHE[key] = _build(flags)
    return _CACHE[key]


def kernel(**inputs):
    B = 8
    flags = {
        "bq": bool(np.any(inputs["bq"])),
        "bk": bool(np.any(inputs["bk"])),
        "bv": bool(np.any(inputs["bv"])),
        "bo": bool(np.any(inputs["bo"])),
        "b1": bool(np.any(inputs["b1"])),
        "b2": bool(np.any(inputs["b2"])),
        "g1": bool(np.any(np.asarray(inputs["g1"]) != 1.0)),
        "be1": bool(np.any(inputs["be1"])),
        "g2": bool(np.any(np.asarray(inputs["g2"]) != 1.0)),
        "be2": bool(np.any(inputs["be2"])),
    }
    nc = _get_nc(flags)

    import ml_dtypes

    bf = ml_dtypes.bfloat16
    shared = {}
    for name in ["Wq", "Wk", "Wv", "Wo", "W1", "W2"]:
        shared[name] = np.ascontiguousarray(
            np.asarray(inputs[name], np.float32).astype(bf)
        )
    for name, on in flags.items():
        if on:
            shared[name] = np.ascontiguousarray(np.asarray(inputs[name], np.float32))

    in_maps = []
    x_full = np.asarray(inputs["x"], np.float32)
    c_full = np.asarray(inputs["cross"], np.float32)
    for core in range(B):
        m = dict(shared)
        m["x"] = np.ascontiguousarray(x_full[core])
        m["xb"] = np.ascontiguousarray(x_full[core].astype(bf))
        m["crossb"] = np.ascontiguousarray(c_full[core].astype(bf))
        in_maps.append(m)

    res = run_bass_kernel_spmd(nc, in_maps, list(range(B)))
    out = np.stack([res.results[i]["out"] for i in range(B)], axis=0)
    A = np.stack([res.results[i]["attn"] for i in range(B)], axis=0)
    return out, A
